# revision 70
# baseline (speedup 1.0000x reference)
"""Multi-head attention forward on 8 Trainium2 NeuronCores.

Problem: batch=8, seq=1024, d_model=1024, n_heads=16, d_head=64, fp32 ref.

Sharding: data-parallel over batch - core b computes batch element b end to
end (weights replicated, no collectives).

Per-core layout strategy (nothing ever needs an on-device transpose):
  - x^T (d on partitions) is staged by the host; it serves as
      rhs  for Q^T/K^T = W^T @ x^T   (2 heads packed -> M=128)
      lhsT for V      = x @ W_V      (heads along the free dim)
  - scores^T = K @ Q^T lands with k on partitions, so softmax's exp is one
    ScalarE activation per tile (the 1/sqrt(d) scale and the key-mask fold
    in as activation scale/bias), and the sum over k happens inside the
    P@V matmul via a ones-column appended to V (softmax denominators pop
    out in psum row 64 for free).
  - Z^T = [V|1]^T @ P^T keeps (head, e) on partitions; heads are packed in
    pairs so the output projection contracts with K=128.
  - biases are folded into the DVE psum->sbuf evacuation ops (per-partition
    tensor_scalar for Q/K, broadcast-staged tensor_tensor for V and the
    output projection) so they cost zero TensorE cycles.

Schedule: one software-pipelined loop over head pairs keeps all engines
busy simultaneously instead of running projection / attention / output
phases back to back:

  pre    : load DMAs, Q^T/K^T projection for pair 0
  iter 0 : scores+exp pair 0 interleaved with the whole V projection,
           then Q^T/K^T pair 1
  iter g : scores+exp pair g interleaved with PV(pair g-1) and the
           Q^T/K^T projection of pair g+1
  tail   : PV(pair 7), output projection

The two heads of a pair have d_head=64 so their score matmuls occupy
disjoint PE row groups (partitions 0-63 / 64-127); emitting them
back-to-back lets the PE run them concurrently (row tiling), and one
N=1024 ScalarE activation then exps both heads' scores at once.
Keeping the PE stream dense also keeps the HAM clock gate at 2.4 GHz
(the phase-serialized version stalled >3.4us on every head and ran the
whole attention phase at the cold 1.2 GHz clock).

PSUM budget (8 banks): 2 proj + 4 scores (2 tiles x 2 banks) + 2 PV.

Everything is bf16 into the PE with fp32 PSUM accumulation.

This toolchain's walrus encodes at most ONE sync wait per instruction, so
_split_multi_waits hoists excess waits onto same-engine EventSemaphore
instructions (engines execute their streams in order, so this is exact).
"""

from contextlib import ExitStack

import numpy as np

import concourse.bass as bass
import concourse.tile as tile
from concourse import mybir
from concourse.bass_utils import run_bass_kernel_spmd

S = 1024  # seq
D = 1024  # d_model
H = 16  # heads
E = 64  # d_head
B = 8  # batch == n_cores
P = 128  # partitions
NS = S // P  # 8 s-tiles
ND = D // P  # 8 d-chunks
NG = H // 2  # 8 head pairs

F32 = mybir.dt.float32
BF16 = mybir.dt.bfloat16
AF = mybir.ActivationFunctionType

MASK_NEG = 60.0  # exp(x - 60) ~ 9e-27: masked keys vanish without inf/nan


def build_program(split_waits=True):
    nc = bass.Bass("TRN2", target_bir_lowering=False, debug=False)

    # all inputs arrive pre-packed by the host into their exact SBUF layouts
    xt_d = nc.dram_tensor("xt", [P, ND, S], BF16, kind="ExternalInput").ap()
    wq_d = nc.dram_tensor("wq", [P, NG, ND, P], BF16, kind="ExternalInput").ap()
    wk_d = nc.dram_tensor("wk", [P, NG, ND, P], BF16, kind="ExternalInput").ap()
    wv_d = nc.dram_tensor("wv", [P, ND, H * E], BF16, kind="ExternalInput").ap()
    wo_d = nc.dram_tensor("wo", [P, NG, D], BF16, kind="ExternalInput").ap()
    # per-partition Q/K biases: col g = b_Q[pair g], col NG+g = b_K[pair g]
    bqk_d = nc.dram_tensor("bqk", [P, 2 * NG], F32, kind="ExternalInput").ap()
    # partition-broadcast b_V (h,e) and b_O (d)
    bvb_d = nc.dram_tensor("bvb", [P, H * E], BF16, kind="ExternalInput").ap()
    bob_d = nc.dram_tensor("bob", [P, D], BF16, kind="ExternalInput").ap()
    mb_d = nc.dram_tensor("mb", [P, NS], F32, kind="ExternalInput").ap()
    # 0/1 selector: sel[32*(2*(g%2)+h2), g, h2*64+e] = 1 — broadcasts the
    # rc row of each (pair, head-half) onto zT's partition layout
    sel_d = nc.dram_tensor("sel", [P, NG, P], BF16, kind="ExternalInput").ap()
    # tail selector: selt[32*(2*h2+qh), qh, h2*64+e] = 1
    selt_d = nc.dram_tensor("selt", [P, 2, P], BF16, kind="ExternalInput").ap()
    out_d = nc.dram_tensor("out", [S, D], F32, kind="ExternalOutput").ap()

    with tile.TileContext(nc) as tc, ExitStack() as ctx:
        g1 = ctx.enter_context(tc.tile_pool(name="g1", bufs=1))
        wqkp = ctx.enter_context(tc.tile_pool(name="wqk", bufs=4))
        ptp = ctx.enter_context(tc.tile_pool(name="ptp", bufs=12))
        obp = ctx.enter_context(tc.tile_pool(name="obp", bufs=2))
        # PSUM: 2 proj + 4 scores + 2 PV/broadcast = 8 banks; the scores
        # pool is closed before the output projection opens its 4-bank pool
        pp = ctx.enter_context(tc.tile_pool(name="pp", bufs=2, space="PSUM"))
        zpsp = ctx.enter_context(tc.tile_pool(name="zps", bufs=2, space="PSUM"))
        st_ctx = ExitStack()
        stp = st_ctx.enter_context(tc.tile_pool(name="stp", bufs=1, space="PSUM"))

        # ---- input DMAs ----
        # xT (the critical 2MB) rides the SP HWDGE ring alone; the weight
        # loads go out in parallel on the Activation HWDGE ring (idle until
        # the first exp). Mid-kernel pair loads go back to the SP ring so
        # they never queue behind exps.
        wqk_t = {}

        def load_pair(g, eng=None):
            wq_t = wqkp.tile([P, ND, P], BF16, tag="wq_t", name=f"wq{g}")
            wk_t = wqkp.tile([P, ND, P], BF16, tag="wk_t", name=f"wk{g}")
            (eng or nc.sync).dma_start(out=wq_t, in_=wq_d[:, g])
            (eng or nc.sync).dma_start(out=wk_t, in_=wk_d[:, g])
            wqk_t[g] = (wq_t, wk_t)

        # first projection's deps (wq0/wk0/xT-lo) lead the SP ring while
        # xT-hi and later weights flow on the Activation ring
        load_pair(0, nc.sync)
        xT = g1.tile([P, ND, S], BF16, tag="xT")
        nc.sync.dma_start(out=xT[:, : ND // 2], in_=xt_d[:, : ND // 2])
        nc.scalar.dma_start(out=xT[:, ND // 2 :], in_=xt_d[:, ND // 2 :])
        load_pair(1, nc.scalar)
        mb_sb = g1.tile([P, NS], F32, tag="mb")
        nc.sync.dma_start(out=mb_sb, in_=mb_d)
        bqk_sb = g1.tile([P, 2 * NG], F32, tag="bqk")
        nc.sync.dma_start(out=bqk_sb, in_=bqk_d)
        wv_sb = g1.tile([P, ND, H * E], BF16, tag="wv_sb")
        nc.scalar.dma_start(out=wv_sb, in_=wv_d)
        bvb_sb = g1.tile([P, H * E], BF16, tag="bvb")
        nc.scalar.dma_start(out=bvb_sb, in_=bvb_d)
        bob_sb = g1.tile([P, D], BF16, tag="bob")
        nc.scalar.dma_start(out=bob_sb, in_=bob_d)
        wo_sb = g1.tile([P, NG, D], BF16, tag="wo_sb")
        nc.scalar.dma_start(out=wo_sb, in_=wo_d)
        sel_sb = g1.tile([P, NG, P], BF16, tag="sel")
        nc.scalar.dma_start(out=sel_sb, in_=sel_d)
        selt_sb = g1.tile([P, 2, P], BF16, tag="selt")
        nc.scalar.dma_start(out=selt_sb, in_=selt_d)

        # persistent activations
        qT = g1.tile([P, NG, S], BF16, tag="qT")
        kT = g1.tile([P, NG, S], BF16, tag="kT")
        vb = g1.tile([P, NS, H, E + 1], BF16, tag="vb")
        zT = g1.tile([P, NG, S], BF16, tag="zT")
        # softmax denominators staged on separate partitions: DVE op cost is
        # per-column regardless of partition count, so one batched
        # reciprocal costs the same ~6.4ns/col as a single-row one (16
        # singles would be 105us of DVE). Engine APs must start at partition
        # 0/32/64/96, so a batch holds the 4 head-halves of 2 pairs.
        # dummy matmuls during the input-DMA wait keep the PE busy so the
        # HAM clock gate is already at 2.4 GHz when real work starts
        warm = g1.tile([P, 512], BF16, tag="warm")
        nc.vector.memset(warm, 0.0)
        warm_ps = zpsp.tile([P, 512], F32, tag="zp", name="warm_ps")
        for _ in range(30):
            nc.tensor.matmul(
                out=warm_ps, lhsT=warm[:, 0:P], rhs=warm, start=True, stop=True
            )

        den_all = g1.tile([P, 2, 512], F32, tag="den_all")
        nc.vector.memset(den_all, 1.0)  # garbage rows must stay finite
        rc_all = g1.tile([P, 3, 2, 512], BF16, tag="rc_all")
        # pairs 6/7 use a half-width batch (qh on partitions as well) so
        # their reciprocal costs 3.3us instead of 6.5 and pair 6's runs
        # during iter 7 — only pair 7's normalize is left for the tail
        den_tl = g1.tile([P, 512], F32, tag="den_tl")
        nc.vector.memset(den_tl, 1.0)
        rc_tl = g1.tile([P, 512], BF16, tag="rc_tl")
        # softmax-sum ones columns (V proj fills cols 0..E-1)
        nc.vector.memset(vb[:, :, :, E : E + 1], 1.0)

        pt_tiles = {}

        qk_ps = {}

        def qk_quarter(g, which, qh, cs):
            # one psum bank per accumulation group (pp bufs=1); the bias
            # folds into the DVE evacuation
            ti = 0 if which == "q" else 1
            dst = qT if which == "q" else kT
            w_t = wqk_t[g][ti]
            bcol = g if which == "q" else NG + g
            if cs[0] == 0:
                qk_ps[(g, ti, qh)] = pp.tile(
                    [P, 512], F32, tag="pp", name=f"qk{g}{ti}{qh}"
                )
            ps = qk_ps[(g, ti, qh)]
            for c in cs:
                nc.tensor.matmul(
                    out=ps,
                    lhsT=w_t[:, c],
                    rhs=xT[:, c, qh * 512 : (qh + 1) * 512],
                    start=(c == 0),
                    stop=(c == ND - 1),
                )
            if cs[-1] == ND - 1:
                nc.vector.tensor_scalar_add(
                    out=dst[:, g, qh * 512 : (qh + 1) * 512],
                    in0=ps,
                    scalar1=bqk_sb[:, bcol : bcol + 1],
                )

        def v_half(st, hh):
            ps = pp.tile([P, 512], F32, tag="pp", name=f"v{st}{hh}")
            for c in range(ND):
                nc.tensor.matmul(
                    out=ps,
                    lhsT=xT[:, c, st * P : (st + 1) * P],
                    rhs=wv_sb[:, c, hh * 512 : (hh + 1) * 512],
                    start=(c == 0),
                    stop=(c == ND - 1),
                )
            nc.vector.tensor_add(
                out=vb[:, st, hh * 8 : (hh + 1) * 8, 0:E],
                in0=ps.rearrange("p (h e) -> p h e", h=8),
                in1=bvb_sb[:, hh * 512 : (hh + 1) * 512].rearrange(
                    "p (h e) -> p h e", h=8
                ),
            )

        def sc(g, kt):
            # the two heads' matmuls sit in disjoint PE row groups -> the
            # qh pairs run concurrently; one N=2048 activation exps the
            # whole (pair, key-tile)
            stt = stp.tile([P, 2, 2, 512], F32, tag="st", name=f"st{g}{kt}")
            ptt = ptp.tile([P, 2, 2, 512], BF16, tag="pt", name=f"pt{g}{kt}")
            for qh in range(2):
                for h2 in range(2):
                    nc.tensor.matmul(
                        out=stt[:, qh, h2],
                        lhsT=kT[h2 * E : (h2 + 1) * E, g, kt * P : (kt + 1) * P],
                        rhs=qT[h2 * E : (h2 + 1) * E, g, qh * 512 : (qh + 1) * 512],
                        start=True,
                        stop=True,
                    )
            nc.scalar.activation(
                out=ptt,
                in_=stt,
                func=AF.Exp,
                bias=mb_sb[:, kt : kt + 1],
                scale=0.125,
            )
            pt_tiles[(g, kt)] = ptt

        pv_zp = {}
        pv_tail = {}

        def pv_mms(g, h2, kts):
            h = 2 * g + h2
            if kts[0] == 0:
                pv_zp[(g, h2)] = [
                    zpsp.tile([P, 512], F32, tag="zp", name=f"zp{g}{h2}{i}")
                    for i in range(2)
                ]
            zp2 = pv_zp[(g, h2)]
            for kt in kts:
                for qh in range(2):  # same lhsT back-to-back
                    nc.tensor.matmul(
                        out=zp2[qh][0 : E + 1],
                        lhsT=vb[:, kt, h, :],
                        rhs=pt_tiles[(g, kt)][:, qh, h2],
                        start=(kt == 0),
                        stop=(kt == NS - 1),
                    )
            if kts[-1] == NS - 1:
                # evacuate PSUM promptly (cheap DVE copies with no DMA
                # dependency) so the next PV group's matmuls never wait on
                # the normalize chain; the reciprocal is ready well before
                # the bc broadcast-matmul scheduled a few chunks later
                with nc.allow_low_precision(reason="bf16 z and softmax denom"):
                    for qh in range(2):
                        # unnormalized z straight into zT (normalized
                        # in place later); denominator row to its own
                        # partition of the batch-reciprocal staging tile
                        nc.vector.tensor_copy(
                            out=zT[
                                h2 * E : (h2 + 1) * E,
                                g,
                                qh * 512 : (qh + 1) * 512,
                            ],
                            in_=zp2[qh][0:E],
                        )
                        if g >= NG - 2:
                            row = 32 * (2 * h2 + qh)
                            dst = den_tl[row : row + 1]
                        else:
                            row = 32 * (2 * (g % 2) + h2)
                            dst = den_all[row : row + 1, qh]
                        nc.vector.tensor_copy(out=dst, in_=zp2[qh][E : E + 1])

        def recip_batch(k):  # batch k = pairs 2k, 2k+1
            with nc.allow_low_precision(reason="bf16 softmax denom"):
                nc.vector.reciprocal(out=rc_all[:, k], in_=den_all)

        def recip_tail():  # half-width batch: one of pairs 6/7
            with nc.allow_low_precision(reason="bf16 softmax denom"):
                nc.vector.reciprocal(out=rc_tl, in_=den_tl)

        def bc_mul(g):
            # normalize one whole pair: a K=128 matmul against a constant
            # 0/1 selector broadcasts the pair's rc rows onto the zT
            # partition layout (h2*64+e), then in-place multiplies
            for qh in range(2):
                bc = zpsp.tile([P, 512], F32, tag="zp", name=f"bc{g}{qh}")
                if g >= NG - 2:
                    lhsT, rhs = selt_sb[:, qh], rc_tl
                else:
                    lhsT, rhs = sel_sb[:, g], rc_all[:, g // 2, qh]
                nc.tensor.matmul(out=bc, lhsT=lhsT, rhs=rhs, start=True, stop=True)
                zt = zT[:, g, qh * 512 : (qh + 1) * 512]
                nc.vector.tensor_mul(zt, zt, bc)

        out_ps = {}

        def out_proj(st, opp, gs):
            if gs[0] == 0:
                out_ps[st] = [
                    opp.tile([P, 512], F32, tag="op", name=f"op{st}{i}")
                    for i in range(2)
                ]
            ops = out_ps[st]
            for g in gs:
                for dh in range(2):  # same lhsT back-to-back
                    nc.tensor.matmul(
                        out=ops[dh],
                        lhsT=zT[:, g, st * P : (st + 1) * P],
                        rhs=wo_sb[:, g, dh * 512 : (dh + 1) * 512],
                        start=(g == 0),
                        stop=(g == NG - 1),
                    )
            if gs[-1] != NG - 1:
                return
            ob = obp.tile([P, D], F32, tag="ob", name=f"ob{st}")
            for dh in range(2):  # per-half DMA, alternating DGE rings
                nc.vector.tensor_add(
                    out=ob[:, dh * 512 : (dh + 1) * 512],
                    in0=ops[dh],
                    in1=bob_sb[:, dh * 512 : (dh + 1) * 512],
                )
                (nc.sync if dh == 0 else nc.scalar).dma_start(
                    out=out_d[st * P : (st + 1) * P, dh * 512 : (dh + 1) * 512],
                    in_=ob[:, dh * 512 : (dh + 1) * 512],
                )

        # ---- pipelined schedule ----
        # pt pool-ring safety with bufs=12: a score unit sc(g, kt) reuses
        # the pt slot of sc(g-1, kt+4); every PV read of that slot is
        # emitted earlier in the iteration (PV chunks precede the score
        # units that recycle their tiles)
        C_LO = list(range(0, ND // 2))
        C_HI = list(range(ND // 2, ND))
        KT_LO = list(range(0, NS // 2))
        KT_HI = list(range(NS // 2, NS))

        def qk4(g, which):  # the four emission chunks of one projection
            # a chunk pair (C_LO then C_HI) must not have other pp-pool
            # allocations between them (pp bufs=1); the main loop only
            # emits sc/pv/bc chunks in between, which use other pools
            return [
                lambda: qk_quarter(g, which, 0, C_LO),
                lambda: qk_quarter(g, which, 0, C_HI),
                lambda: qk_quarter(g, which, 1, C_LO),
                lambda: qk_quarter(g, which, 1, C_HI),
            ]

        def qk_full(g, which, qh):  # whole accumulation group as one chunk
            return lambda: qk_quarter(g, which, qh, list(range(ND)))

        for which in ("q", "k"):
            for qh in range(2):
                qk_full(0, which, qh)()
        load_pair(2)
        # iter 0: scores pair 0 + whole V projection + Q/K of pair 1
        # (full groups only: v_half also allocates from the 1-deep pp pool)
        qk1 = [qk_full(1, w, qh) for w in ("q", "k") for qh in range(2)]
        for kt in range(NS):
            sc(0, kt)
            v_half(kt, 0)
            v_half(kt, 1)
            if kt % 2 == 1:
                qk1[kt // 2]()
        for g in range(1, NG):
            if g + 2 <= NG - 1:
                load_pair(g + 2)
            last = g == NG - 1
            nxt = qk4(g + 1, "q") + qk4(g + 1, "k") if not last else None
            pv_mms(g - 1, 0, KT_LO)
            sc(g, 0)
            pv_mms(g - 1, 0, KT_HI)  # ends: z/denominator evacuation
            sc(g, 1)
            if nxt:
                nxt[0]()
            sc(g, 2)
            if nxt:
                nxt[1]()
            sc(g, 3)
            pv_mms(g - 1, 1, KT_LO)
            sc(g, 4)
            pv_mms(g - 1, 1, KT_HI)
            if g % 2 == 0:
                # pairs g-2, g-1 fully staged: one batched reciprocal
                recip_batch((g - 2) // 2)
            if last:
                recip_tail()  # pair 6, staged in the half-width batch
            sc(g, 5)
            if nxt:
                nxt[2]()
            sc(g, 6)
            if g >= 3 and g % 2 == 1:  # normalize the batch recip'd last iter
                bc_mul(g - 3)
            if nxt:
                nxt[3]()
            if last:
                pv_mms(NG - 1, 0, KT_LO)  # chases exp(7, kt<4), done by now
            sc(g, 7)
            if g >= 3 and g % 2 == 1:
                bc_mul(g - 2)
            if nxt:
                for f in nxt[4:]:
                    f()
        # tail: the rest of PV(7) chases its exps; its denominator copies
        # overwrite the half-width batch (pair 6's reciprocal already read
        # it — DVE is in-order), then one 3.3us reciprocal remains.
        # bc_mul(6) sits after pv(7,0,HI) so its psum-ring slots recycle
        # banks whose accumulation groups have already been evacuated.
        pv_mms(NG - 1, 0, KT_HI)
        bc_mul(NG - 2)  # pair 6 (recip_tail ran several chunks ago)
        pv_mms(NG - 1, 1, KT_LO)
        pv_mms(NG - 1, 1, KT_HI)
        recip_tail()  # pair 7
        # free the 4 score banks, then the output projection double-buffers
        # its two psum halves there
        st_ctx.close()
        G_DONE = list(range(NG - 1))  # pairs already normalized
        G_LAST = [NG - 1]
        with tc.tile_pool(name="opp", bufs=4, space="PSUM") as opp:
            # the first two tiles' pair-0..6 matmuls run under the final
            # reciprocal; pair 7 contributions come after its normalize
            out_proj(0, opp, G_DONE)
            out_proj(1, opp, G_DONE)
            bc_mul(NG - 1)
            out_proj(0, opp, G_LAST)
            out_proj(1, opp, G_LAST)
            for st in range(2, NS):
                out_proj(st, opp, list(range(NG)))

    if split_waits:
        _split_multi_waits(nc)
    return nc


def _split_multi_waits(nc):
    """This walrus build encodes at most ONE sync wait per instruction.
    Tile emits more. Hoist excess waits onto same-engine EventSemaphore
    instructions inserted immediately before the offender - engines and
    DGE sequencers execute their streams in order, so this preserves
    semantics exactly."""
    n = 0
    for fn in nc.m.functions:
        for bb in fn.blocks:
            out = []
            for inst in bb.instructions:
                si = getattr(inst, "sync_info", None)
                waits = list(si.on_wait) if si is not None and si.on_wait else []
                if len(waits) > 1:
                    for w in waits[:-1]:
                        n += 1
                        out.append(
                            mybir.InstEventSemaphore(
                                name=f"evw-{n}",
                                engine=inst.engine,
                                sync_info=mybir.SyncInfo(
                                    on_wait=[w], on_update=[]
                                ),
                            )
                        )
                    si.on_wait = [waits[-1]]
                out.append(inst)
            bb.instructions[:] = out


_NC_CACHE = None


def _get_nc():
    global _NC_CACHE
    if _NC_CACHE is None:
        _NC_CACHE = build_program()
    return _NC_CACHE


def _make_in_maps(inputs):
    import ml_dtypes

    bf16 = ml_dtypes.bfloat16
    x = np.asarray(inputs["x"], np.float32)
    mask = np.asarray(inputs["key_attention_mask"])
    wq = np.asarray(inputs["W_Q"], np.float32).astype(bf16)
    wk = np.asarray(inputs["W_K"], np.float32).astype(bf16)
    wv = np.asarray(inputs["W_V"], np.float32).astype(bf16)
    wo = np.asarray(inputs["W_O"], np.float32).astype(bf16)
    bq = np.asarray(inputs["b_Q"], np.float32)  # (H, E)
    bk = np.asarray(inputs["b_K"], np.float32)
    bv = np.asarray(inputs["b_V"], np.float32)
    bo = np.asarray(inputs["b_O"], np.float32)  # (D,)

    def pack_qk(w):  # (H, D, E) -> [p, g, c, (h2 e)]
        return np.ascontiguousarray(
            w.reshape(NG, 2, ND, P, E).transpose(3, 0, 2, 1, 4).reshape(P, NG, ND, P)
        )

    # per-partition (half*64+e) bias columns per pair
    def pack_b(b):  # (H, E) -> [p, g]
        return np.ascontiguousarray(
            b.reshape(NG, 2, E).transpose(1, 2, 0).reshape(P, NG)
        )

    bqk = np.concatenate([pack_b(bq), pack_b(bk)], axis=1).astype(np.float32)
    sel = np.zeros((P, NG, P), np.float32)
    for g in range(NG):
        for h2 in range(2):
            sel[32 * (2 * (g % 2) + h2), g, h2 * E : (h2 + 1) * E] = 1.0
    selt = np.zeros((P, 2, P), np.float32)
    for qh in range(2):
        for h2 in range(2):
            selt[32 * (2 * h2 + qh), qh, h2 * E : (h2 + 1) * E] = 1.0
    shared = {
        "sel": sel.astype(bf16),
        "selt": selt.astype(bf16),
        "wq": pack_qk(wq),
        "wk": pack_qk(wk),
        # (H, D, E) -> [p, c, (h e)]
        "wv": np.ascontiguousarray(
            wv.reshape(H, ND, P, E).transpose(2, 1, 0, 3).reshape(P, ND, H * E)
        ),
        # (H, E, D) -> [(h2 e), g, d]
        "wo": np.ascontiguousarray(
            wo.reshape(NG, 2, E, D).transpose(1, 2, 0, 3).reshape(P, NG, D)
        ),
        "bqk": bqk,
        "bvb": np.ascontiguousarray(
            np.tile(bv.reshape(1, H * E), (P, 1))
        ).astype(bf16),
        "bob": np.ascontiguousarray(np.tile(bo.reshape(1, D), (P, 1))).astype(
            bf16
        ),
    }
    in_maps = []
    for b in range(B):
        m = dict(shared)
        xt = x[b].T.astype(bf16)  # (D, S) -> [p, c, s]
        m["xt"] = np.ascontiguousarray(
            xt.reshape(ND, P, S).transpose(1, 0, 2)
        )
        mb = ((mask[b] != 0).astype(np.float32) - 1.0) * MASK_NEG
        m["mb"] = np.ascontiguousarray(mb.reshape(NS, P).T)
        in_maps.append(m)
    return in_maps


def run(inputs, trace=False):
    nc = _get_nc()
    res = run_bass_kernel_spmd(nc, _make_in_maps(inputs), list(range(B)),
                               trace=trace)
    out = np.stack([res.results[b]["out"] for b in range(B)], axis=0)
    return out, res


def kernel(**inputs) -> np.ndarray:
    out, _ = run(inputs, trace=False)
    return out
